# revision 14
# baseline (speedup 1.0000x reference)
"""Trainium2 Bass kernel for nn_Attention_4501125726440 (sparse_attention).

Full attention layer: QKV projections, per-head-dim RMSNorm on Q/K, full-head
RoPE, causal attention with sink-augmented softmax, output projection.

Sharding: 8 cores = (batch b in {0,1}) x (head-group hg in {0..3}, 4 heads
each).  Each core computes its batch's 4 heads end-to-end plus the partial
output projection through the matching 256 rows of wo^T; the host sums the 4
partials per batch (row-parallel tensor parallelism).

Device layout (B=2, S=2048, D=1024, H=16, HD=64; per core: 4 heads):
  - x is fed host-transposed as xT [D, S]; Q/K are computed feature-major
    ([256 feats on partitions, S free]) so the attention score matmuls need
    no transposes at all; V is computed sequence-major from lhsT = xT tiles.
  - RMSNorm in feature-major uses two tiny matmuls: a block-diagonal ones
    matrix reduces sum-of-squares across each head's 64 partitions, and its
    transpose (scaled by the norm weight and HD**-0.25 of the softmax scale)
    broadcasts the reciprocal RMS back across partitions.
  - RoPE's rotate-half is a 128x128 signed permutation matmul (cos/sin share
    values between feature d and d+32, so rot(q) * sin == rot(q * sin)).
  - Attention computes transposed score blocks sT[k,q] = K-block^T @ Q so exp
    applies block-wise and the P@V matmul consumes them directly (lhsT = V
    block).  V carries an all-ones 65th column, so row 64 of the PV psum
    accumulator is the softmax denominator for free.  No max-subtraction
    needed: post-RMSNorm |q|,|k| <= 8*max|w| keeps |scores| <= ~8.
  - The denominator row (+ exp(sink)) is reciprocated and broadcast back
    across the 64 head partitions with a K=1 outer-product matmul.

Matmul inputs are bf16 (cast on host); all accumulation is fp32 in PSUM.
"""

import sys

import ml_dtypes
import numpy as np

_REPO = "/opt/trn_rl_repo"
if _REPO not in sys.path:
    sys.path.insert(0, _REPO)

import concourse.bacc as bacc  # noqa: E402
import concourse.mybir as mybir  # noqa: E402
import concourse.tile as tile  # noqa: E402
from concourse.bass_utils import run_bass_kernel_spmd  # noqa: E402
from concourse.masks import make_identity  # noqa: E402

B, S, D = 2, 2048, 1024
H = 16
HD = 64
HEADS_PER_CORE = 4
FEATS = HEADS_PER_CORE * HD  # 256
EPS = 1e-6
ROPE_BASE = 10000.0
N_CORES = 8

F32 = mybir.dt.float32
BF16 = mybir.dt.bfloat16
BF16_NP = ml_dtypes.bfloat16

DCH = D // 128      # 8 contraction chunks for projections
FCH = FEATS // 128  # 2 feature chunks (2 heads each)
SQ = 512            # q-tile width in attention / projection free chunks
NSQ = S // SQ       # 4
NKB = S // 128      # 16 key blocks
NQT = S // 128      # 16 128-row q tiles

EXP = mybir.ActivationFunctionType.Exp
SQRT = mybir.ActivationFunctionType.Sqrt


def build_program():
    nc = bacc.Bacc("TRN2", target_bir_lowering=False, debug=False)

    xT = nc.dram_tensor("xT", [D, S], BF16, kind="ExternalInput").ap()
    wqT = nc.dram_tensor("wqT", [D, FEATS], BF16, kind="ExternalInput").ap()
    wkT = nc.dram_tensor("wkT", [D, FEATS], BF16, kind="ExternalInput").ap()
    wvT = nc.dram_tensor("wvT", [D, FEATS], BF16, kind="ExternalInput").ap()
    woT = nc.dram_tensor("woT", [FEATS, D], BF16, kind="ExternalInput").ap()
    cosT = nc.dram_tensor("cosT", [128, S], BF16, kind="ExternalInput").ap()
    sinT = nc.dram_tensor("sinT", [128, S], BF16, kind="ExternalInput").ap()
    trimask = nc.dram_tensor("trimask", [128, 128], BF16, kind="ExternalInput").ap()
    bd1 = nc.dram_tensor("bd1", [128, 2], BF16, kind="ExternalInput").ap()
    qbd = nc.dram_tensor("qbd", [2, 128], BF16, kind="ExternalInput").ap()
    kbd = nc.dram_tensor("kbd", [2, 128], BF16, kind="ExternalInput").ap()
    rotm = nc.dram_tensor("rotm", [128, 128], BF16, kind="ExternalInput").ap()
    sinkexp = nc.dram_tensor("sinkexp", [1, HEADS_PER_CORE], F32, kind="ExternalInput").ap()
    y = nc.dram_tensor("y", [S, D], F32, kind="ExternalOutput").ap()

    xT3 = xT.rearrange("(o p) s -> p o s", p=128)      # [128, 8, S]
    wqT3 = wqT.rearrange("(o p) f -> p o f", p=128)    # [128, 8, 256]
    wkT3 = wkT.rearrange("(o p) f -> p o f", p=128)
    wvT3 = wvT.rearrange("(o p) f -> p o f", p=128)
    woT3 = woT.rearrange("(o p) d -> p o d", p=128)    # [128, 2, 1024]

    with tile.TileContext(nc) as tc, nc.allow_low_precision(reason="bf16 matmul pipeline"):
        with (
            tc.tile_pool(name="persist", bufs=1) as persist,
            tc.tile_pool(name="consts", bufs=1) as consts,
        ):
            # Persistent SBUF tensors
            q_sb = persist.tile([128, FCH, S], BF16, tag="q_sb")
            k_sb = persist.tile([128, FCH, S], BF16, tag="k_sb")
            # V per head, with an appended ones column (65th) for denominators
            v_sb = [persist.tile([128, NKB, HD + 1], BF16, tag=f"v_sb{h}",
                                 name=f"v_sb{h}")
                    for h in range(HEADS_PER_CORE)]
            ot_sb = persist.tile([128, FCH, S], BF16, tag="ot_sb")

            cos_sb = consts.tile([128, S], BF16, tag="cos_sb")
            sin_sb = consts.tile([128, S], BF16, tag="sin_sb")
            mask_sb = consts.tile([128, 128], BF16, tag="mask_sb")
            bd1_sb = consts.tile([128, 2], BF16, tag="bd1_sb")
            qbd_sb = consts.tile([2, 128], BF16, tag="qbd_sb")
            kbd_sb = consts.tile([2, 128], BF16, tag="kbd_sb")
            rotm_sb = consts.tile([128, 128], BF16, tag="rotm_sb")
            sink_sb = consts.tile([1, HEADS_PER_CORE], F32, tag="sink_sb")
            ones_sb = consts.tile([1, HD], BF16, tag="ones_sb")
            ident_sb = consts.tile([128, 128], BF16, tag="ident_sb")
            eps_sb = consts.tile([128, 1], F32, tag="eps_sb")

            nc.gpsimd.dma_start(cos_sb[:], cosT)
            nc.gpsimd.dma_start(sin_sb[:], sinT)
            nc.gpsimd.dma_start(mask_sb[:], trimask)
            nc.gpsimd.dma_start(bd1_sb[:], bd1)
            nc.gpsimd.dma_start(qbd_sb[:], qbd)
            nc.gpsimd.dma_start(kbd_sb[:], kbd)
            nc.gpsimd.dma_start(rotm_sb[:], rotm)
            nc.gpsimd.dma_start(sink_sb[:], sinkexp)
            nc.vector.memset(ones_sb[:], 1.0)
            make_identity(nc, ident_sb[:])
            nc.vector.memset(eps_sb[:], EPS)
            for h in range(HEADS_PER_CORE):
                nc.vector.memset(v_sb[h][:, :, HD:HD + 1], 1.0)

            # ---------------- Phase 1: QKV projections -----------------
            with (
                tc.tile_pool(name="p1", bufs=2) as p1,
                tc.tile_pool(name="p1x", bufs=1) as p1x,
                tc.tile_pool(name="p1work", bufs=3) as p1w,
                tc.tile_pool(name="p1ps", bufs=4, space="PSUM") as p1ps,
                tc.tile_pool(name="p1ps_small", bufs=2, space="PSUM") as p1pss,
            ):
                xt_sb = p1x.tile([128, DCH, S], BF16, tag="xt_sb")
                wq_sb0 = p1.tile([128, DCH, FEATS], BF16, tag="w_sb", name="wq_sb")
                nc.sync.dma_start(wq_sb0[:], wqT3)
                for d in range(DCH):
                    nc.sync.dma_start(xt_sb[:, d, :], xT3[:, d, :])

                def qk_proj(wT3_ap, bd_w_sb, dst_sb, wtag):
                    if wT3_ap is None:
                        w_sb = wq_sb0
                    else:
                        w_sb = p1.tile([128, DCH, FEATS], BF16, tag="w_sb", name=wtag)
                        nc.sync.dma_start(w_sb[:], wT3_ap)
                    for f in range(FCH):
                        pss = []
                        for s in range(NSQ):
                            ps = p1ps.tile([128, SQ], F32, tag="qkv_ps")
                            pss.append(ps)
                        for d in range(DCH):
                            for s in range(NSQ):
                                nc.tensor.matmul(
                                    pss[s][:],
                                    w_sb[:, d, f * 128:(f + 1) * 128],
                                    xt_sb[:, d, s * SQ:(s + 1) * SQ],
                                    start=(d == 0),
                                    stop=(d == DCH - 1),
                                )
                        for s in range(NSQ):
                            ps = pss[s]
                            raw = p1w.tile([128, SQ], BF16, tag="raw")
                            sq = p1w.tile([128, SQ], BF16, tag="sq")
                            nc.scalar.copy(raw[:], ps[:])
                            nc.scalar.square(sq[:], ps[:])
                            ss_ps = p1pss.tile([128, SQ], F32, tag="sml_ps",
                                               name="ss_ps")[0:2]
                            nc.tensor.matmul(ss_ps[:], bd1_sb[:], sq[:],
                                             start=True, stop=True)
                            rms = p1w.tile([2, SQ], F32, tag="rms")
                            nc.scalar.activation(
                                rms[:], ss_ps[:], SQRT,
                                bias=eps_sb[0:2, :], scale=1.0 / HD,
                            )
                            rinv = p1w.tile([2, SQ], F32, tag="rinv")
                            nc.vector.reciprocal_approx_fast(rinv[:], rms[:])
                            rmsb = p1w.tile([2, SQ], BF16, tag="rmsb")
                            nc.vector.tensor_copy(rmsb[:], rinv[:])
                            bsc_ps = p1pss.tile([128, SQ], F32, tag="sml_ps",
                                                name="bsc_ps")
                            nc.tensor.matmul(bsc_ps[:], bd_w_sb[:], rmsb[:],
                                             start=True, stop=True)
                            qn = p1w.tile([128, SQ], BF16, tag="qn")
                            nc.vector.tensor_mul(qn[:], raw[:], bsc_ps[:])
                            # RoPE: rotate-half via 32-partition shifted copies
                            qs = p1w.tile([128, SQ], BF16, tag="qs")
                            nc.vector.tensor_mul(
                                qs[:], qn[:], sin_sb[:, s * SQ:(s + 1) * SQ])
                            rot = p1w.tile([128, SQ], BF16, tag="rot")
                            for base in (0, 64):
                                nc.vector.tensor_scalar_mul(
                                    rot[base:base + 32, :],
                                    qs[base + 32:base + 64, :], -1.0)
                                nc.vector.tensor_copy(
                                    rot[base + 32:base + 64, :],
                                    qs[base:base + 32, :])
                            qc = p1w.tile([128, SQ], BF16, tag="qc")
                            nc.vector.tensor_mul(
                                qc[:], qn[:], cos_sb[:, s * SQ:(s + 1) * SQ])
                            nc.vector.tensor_add(
                                dst_sb[:, f, s * SQ:(s + 1) * SQ], qc[:], rot[:])

                qk_proj(None, qbd_sb, q_sb, "wq_sb")
                qk_proj(wkT3, kbd_sb, k_sb, "wk_sb")

                # V projection: feature-major like Q/K (fatter matmuls),
                # then PE-transposed into sequence-major V blocks.
                wv_sb = p1.tile([128, DCH, FEATS], BF16, tag="w_sb", name="wv_sb")
                nc.sync.dma_start(wv_sb[:], wvT3)
                for f in range(FCH):
                    for s in range(NSQ):
                        ps = p1ps.tile([128, SQ], F32, tag="qkv_ps", name="v_ps")
                        for d in range(DCH):
                            nc.tensor.matmul(
                                ps[:],
                                wv_sb[:, d, f * 128:(f + 1) * 128],
                                xt_sb[:, d, s * SQ:(s + 1) * SQ],
                                start=(d == 0),
                                stop=(d == DCH - 1),
                            )
                        vf = p1w.tile([128, SQ], BF16, tag="vf")
                        nc.scalar.copy(vf[:], ps[:])
                        for b in range(SQ // 128):
                            qt = s * (SQ // 128) + b
                            tr_ps = p1pss.tile([128, SQ], BF16, tag="sml_ps",
                                               name="tr_ps")[:, 0:128]
                            nc.tensor.transpose(
                                tr_ps[:], vf[:, b * 128:(b + 1) * 128],
                                ident_sb[:])
                            for hh in range(2):
                                h = 2 * f + hh
                                nc.scalar.copy(
                                    v_sb[h][:, qt, 0:HD],
                                    tr_ps[:, hh * HD:(hh + 1) * HD])

            # ------------- Phase 2+3: attention + output projection ------
            # t (512-wide q-range) is the outer loop; once both head-pair
            # chunks finish a q-range, its slice of the output projection is
            # issued, overlapping the next tile's attention and keeping the
            # PE fed with full-array matmuls.
            with (
                tc.tile_pool(name="p2p", bufs=4) as p2p,
                tc.tile_pool(name="p2w", bufs=2) as p2w,
                tc.tile_pool(name="p3", bufs=1) as p3,
                tc.tile_pool(name="p3w", bufs=3) as p3w,
                tc.tile_pool(name="p2s_ps", bufs=2, space="PSUM") as p2sps,
                tc.tile_pool(name="p2o_ps", bufs=3, space="PSUM") as p2ops,
                tc.tile_pool(name="p3ps", bufs=1, space="PSUM") as p3ps,
            ):
                wo_sb = p3.tile([128, FCH, D], BF16, tag="wo_sb")
                nc.gpsimd.dma_start(wo_sb[:], woT3)
                for t in range(NSQ):
                    nkb = (t + 1) * (SQ // 128)  # causal: key blocks 0..nkb-1
                    for f in range(FCH):
                        ot_ps = [
                            p2ops.tile([HD + 1, SQ], F32, tag="ot_ps",
                                       name=f"ot_ps{hh}")
                            for hh in range(2)
                        ]
                        for kb in range(nkb):
                            j = kb - (t * (SQ // 128))
                            qlo = max(j, 0) * 128
                            sp = p2sps.tile([128, 2 * SQ], F32, tag="s_ps")
                            for hh in range(2):
                                plo = hh * HD
                                nc.tensor.matmul(
                                    sp[:, hh * SQ + qlo:(hh + 1) * SQ],
                                    k_sb[plo:plo + HD, f, kb * 128:(kb + 1) * 128],
                                    q_sb[plo:plo + HD, f, t * SQ + qlo:(t + 1) * SQ],
                                    start=True, stop=True,
                                )
                            p_sb = p2p.tile([128, 2 * SQ], BF16, tag="p_sb")
                            if qlo == 0:
                                nc.scalar.activation(p_sb[:], sp[:], EXP)
                            else:
                                for hh in range(2):
                                    nc.scalar.activation(
                                        p_sb[:, hh * SQ + qlo:(hh + 1) * SQ],
                                        sp[:, hh * SQ + qlo:(hh + 1) * SQ], EXP)
                            if j >= 0:
                                for hh in range(2):
                                    nc.vector.tensor_mul(
                                        p_sb[:, hh * SQ + qlo:hh * SQ + qlo + 128],
                                        p_sb[:, hh * SQ + qlo:hh * SQ + qlo + 128],
                                        mask_sb[:])
                            for hh in range(2):
                                h = 2 * f + hh
                                nc.tensor.matmul(
                                    ot_ps[hh][:, qlo:],
                                    v_sb[h][:, kb, :],
                                    p_sb[:, hh * SQ + qlo:(hh + 1) * SQ],
                                    start=(kb == 0),
                                    stop=(kb == nkb - 1),
                                )
                        # denominators: row HD of ot_ps (+ exp(sink)).  The two
                        # copies run first so ot_ps frees for the next tile.
                        dens, ot_us = [], []
                        for hh in range(2):
                            den = p2w.tile([1, SQ], F32, tag="den",
                                           name=f"den{hh}")
                            nc.vector.tensor_copy(den[:], ot_ps[hh][HD:HD + 1, :])
                            ot_u = p2w.tile([HD, SQ], BF16, tag="ot_u",
                                            name=f"ot_u{hh}")
                            nc.vector.tensor_copy(ot_u[:], ot_ps[hh][0:HD, :])
                            dens.append(den)
                            ot_us.append(ot_u)
                        for hh in range(2):
                            h = 2 * f + hh
                            plo = hh * HD
                            den, ot_u = dens[hh], ot_us[hh]
                            nc.vector.tensor_scalar_add(
                                den[:], den[:], sink_sb[0:1, h:h + 1])
                            nc.vector.reciprocal_approx_fast(den[:], den[:])
                            bc_sb = p2w.tile([HD, SQ], F32, tag="bc_sb",
                                             name=f"bc_sb{hh}")
                            nc.gpsimd.partition_broadcast(bc_sb[:], den[:])
                            nc.vector.tensor_mul(
                                ot_sb[plo:plo + HD, f, t * SQ:(t + 1) * SQ],
                                ot_u[:], bc_sb[:])
                    # output projection for this q-range (4 x 128 rows)
                    for qi in range(SQ // 128):
                        qt = t * (SQ // 128) + qi
                        y_sb = p3w.tile([128, D], F32, tag="y_sb")
                        for n in range(D // SQ):
                            y_ps = p3ps.tile([128, SQ], F32, tag="y_ps")
                            for c in range(FCH):
                                nc.tensor.matmul(
                                    y_ps[:],
                                    ot_sb[:, c, qt * 128:(qt + 1) * 128],
                                    wo_sb[:, c, n * SQ:(n + 1) * SQ],
                                    start=(c == 0),
                                    stop=(c == FCH - 1),
                                )
                            nc.scalar.copy(y_sb[:, n * SQ:(n + 1) * SQ], y_ps[:])
                        nc.sync.dma_start(y[qt * 128:(qt + 1) * 128, :], y_sb[:])

    nc.compile()
    return nc


_NC_CACHE = None


def _get_program():
    global _NC_CACHE
    if _NC_CACHE is None:
        _NC_CACHE = build_program()
    return _NC_CACHE


def _b(x):
    return np.ascontiguousarray(np.asarray(x, dtype=np.float32)).astype(BF16_NP)


def _host_inputs(x, wq, wk, wv, wo, q_norm_w, k_norm_w, sink_logit):
    """Build the 8 per-core input maps."""
    x = np.asarray(x, dtype=np.float32)
    wq = np.asarray(wq, dtype=np.float32)
    wk = np.asarray(wk, dtype=np.float32)
    wv = np.asarray(wv, dtype=np.float32)
    wo = np.asarray(wo, dtype=np.float32)
    q_norm_w = np.asarray(q_norm_w, dtype=np.float32)
    k_norm_w = np.asarray(k_norm_w, dtype=np.float32)
    sink_logit = np.asarray(sink_logit, dtype=np.float32)

    # rope tables, feature-major, duplicated across the two heads per chunk
    inv_freq = 1.0 / (ROPE_BASE ** (np.arange(0, HD, 2, dtype=np.float32) / HD))
    tpos = np.arange(S, dtype=np.float32)
    freqs = tpos[:, None] * inv_freq[None, :]           # [S, 32]
    emb = np.concatenate([freqs, freqs], axis=-1)       # [S, 64]
    cosT = _b(np.tile(np.cos(emb).T, (2, 1)))           # [128, S]
    sinT = _b(np.tile(np.sin(emb).T, (2, 1)))

    # triangular causal mask for the single diagonal 128-wide band
    kk = np.arange(128)[:, None]
    qq = np.arange(128)[None, :]
    trimask = _b((kk <= qq).astype(np.float32))          # [128, 128]

    bd1 = np.zeros((128, 2), dtype=np.float32)
    bd1[0:64, 0] = 1.0
    bd1[64:128, 1] = 1.0
    bd1 = _b(bd1)

    scale_half = float(HD) ** -0.25  # sqrt of softmax scale, folded into q & k
    qbd = np.zeros((2, 128), dtype=np.float32)
    kbd = np.zeros((2, 128), dtype=np.float32)
    for m in range(2):
        qbd[m, m * 64:(m + 1) * 64] = q_norm_w * scale_half
        kbd[m, m * 64:(m + 1) * 64] = k_norm_w * scale_half
    qbd = _b(qbd)
    kbd = _b(kbd)

    rotm = np.zeros((128, 128), dtype=np.float32)
    for base in (0, 64):
        for m in range(32):
            rotm[base + m + 32, base + m] = -1.0
            rotm[base + m, base + m + 32] = 1.0
    rotm = _b(rotm)

    in_maps = []
    xT_b = [_b(x[b].T) for b in range(B)]
    for core in range(N_CORES):
        b = core // 4
        hg = core % 4
        rows = slice(hg * FEATS, (hg + 1) * FEATS)
        heads = slice(hg * HEADS_PER_CORE, (hg + 1) * HEADS_PER_CORE)
        in_maps.append({
            "xT": xT_b[b],
            "wqT": _b(wq[rows, :].T),
            "wkT": _b(wk[rows, :].T),
            "wvT": _b(wv[rows, :].T),
            "woT": _b(wo[:, rows].T),
            "cosT": cosT,
            "sinT": sinT,
            "trimask": trimask,
            "bd1": bd1,
            "qbd": qbd,
            "kbd": kbd,
            "rotm": rotm,
            "sinkexp": np.exp(sink_logit[heads]).astype(np.float32).reshape(
                1, HEADS_PER_CORE),
        })
    return in_maps


def kernel(x, wq, wk, wv, wo, q_norm_w, k_norm_w, sink_logit, _run_kwargs=None):
    nc = _get_program()
    in_maps = _host_inputs(x, wq, wk, wv, wo, q_norm_w, k_norm_w, sink_logit)
    res = run_bass_kernel_spmd(nc, in_maps, core_ids=list(range(N_CORES)),
                               **(_run_kwargs or {}))
    out = np.zeros((B, S, D), dtype=np.float32)
    for core in range(N_CORES):
        out[core // 4] += res.results[core]["y"]
    if _run_kwargs:
        kernel.last_result = res
    return out


# revision 15
# speedup vs baseline: 1.0542x; 1.0542x over previous
"""Trainium2 Bass kernel for nn_Attention_4501125726440 (sparse_attention).

Full attention layer: QKV projections, per-head-dim RMSNorm on Q/K, full-head
RoPE, causal attention with sink-augmented softmax, output projection.

Sharding: 8 cores = (batch b in {0,1}) x (head-group hg in {0..3}, 4 heads
each).  Each core computes its batch's 4 heads end-to-end plus the partial
output projection through the matching 256 rows of wo^T; the host sums the 4
partials per batch (row-parallel tensor parallelism).

Device layout (B=2, S=2048, D=1024, H=16, HD=64; per core: 4 heads):
  - x is fed host-transposed as xT [D, S]; Q/K are computed feature-major
    ([256 feats on partitions, S free]) so the attention score matmuls need
    no transposes at all; V is computed sequence-major from lhsT = xT tiles.
  - RMSNorm in feature-major uses two tiny matmuls: a block-diagonal ones
    matrix reduces sum-of-squares across each head's 64 partitions, and its
    transpose (scaled by the norm weight and HD**-0.25 of the softmax scale)
    broadcasts the reciprocal RMS back across partitions.
  - RoPE's rotate-half is a 128x128 signed permutation matmul (cos/sin share
    values between feature d and d+32, so rot(q) * sin == rot(q * sin)).
  - Attention computes transposed score blocks sT[k,q] = K-block^T @ Q so exp
    applies block-wise and the P@V matmul consumes them directly (lhsT = V
    block).  V carries an all-ones 65th column, so row 64 of the PV psum
    accumulator is the softmax denominator for free.  No max-subtraction
    needed: post-RMSNorm |q|,|k| <= 8*max|w| keeps |scores| <= ~8.
  - The denominator row (+ exp(sink)) is reciprocated and broadcast back
    across the 64 head partitions with a K=1 outer-product matmul.

Matmul inputs are bf16 (cast on host); all accumulation is fp32 in PSUM.
"""

import sys

import ml_dtypes
import numpy as np

_REPO = "/opt/trn_rl_repo"
if _REPO not in sys.path:
    sys.path.insert(0, _REPO)

import concourse.bacc as bacc  # noqa: E402
import concourse.mybir as mybir  # noqa: E402
import concourse.tile as tile  # noqa: E402
from concourse.bass_utils import run_bass_kernel_spmd  # noqa: E402
from concourse.masks import make_identity  # noqa: E402

B, S, D = 2, 2048, 1024
H = 16
HD = 64
HEADS_PER_CORE = 4
FEATS = HEADS_PER_CORE * HD  # 256
EPS = 1e-6
ROPE_BASE = 10000.0
N_CORES = 8

F32 = mybir.dt.float32
BF16 = mybir.dt.bfloat16
BF16_NP = ml_dtypes.bfloat16

DCH = D // 128      # 8 contraction chunks for projections
FCH = FEATS // 128  # 2 feature chunks (2 heads each)
SQ = 512            # q-tile width in attention / projection free chunks
NSQ = S // SQ       # 4
NKB = S // 128      # 16 key blocks
NQT = S // 128      # 16 128-row q tiles

EXP = mybir.ActivationFunctionType.Exp
SQRT = mybir.ActivationFunctionType.Sqrt


def build_program():
    nc = bacc.Bacc("TRN2", target_bir_lowering=False, debug=False)

    xT = nc.dram_tensor("xT", [D, S], BF16, kind="ExternalInput").ap()
    wqT = nc.dram_tensor("wqT", [D, FEATS], BF16, kind="ExternalInput").ap()
    wkT = nc.dram_tensor("wkT", [D, FEATS], BF16, kind="ExternalInput").ap()
    wvT = nc.dram_tensor("wvT", [D, FEATS], BF16, kind="ExternalInput").ap()
    woT = nc.dram_tensor("woT", [FEATS, D], BF16, kind="ExternalInput").ap()
    cosT = nc.dram_tensor("cosT", [128, S], BF16, kind="ExternalInput").ap()
    sinT = nc.dram_tensor("sinT", [128, S], BF16, kind="ExternalInput").ap()
    trimask = nc.dram_tensor("trimask", [128, 128], BF16, kind="ExternalInput").ap()
    bd1 = nc.dram_tensor("bd1", [128, 2], BF16, kind="ExternalInput").ap()
    qbd = nc.dram_tensor("qbd", [2, 128], BF16, kind="ExternalInput").ap()
    kbd = nc.dram_tensor("kbd", [2, 128], BF16, kind="ExternalInput").ap()
    rotm = nc.dram_tensor("rotm", [128, 128], BF16, kind="ExternalInput").ap()
    sinkexp = nc.dram_tensor("sinkexp", [1, HEADS_PER_CORE], F32, kind="ExternalInput").ap()
    y = nc.dram_tensor("y", [S, D], F32, kind="ExternalOutput").ap()

    xT3 = xT.rearrange("(o p) s -> p o s", p=128)      # [128, 8, S]
    wqT3 = wqT.rearrange("(o p) f -> p o f", p=128)    # [128, 8, 256]
    wkT3 = wkT.rearrange("(o p) f -> p o f", p=128)
    wvT3 = wvT.rearrange("(o p) f -> p o f", p=128)
    woT3 = woT.rearrange("(o p) d -> p o d", p=128)    # [128, 2, 1024]

    with tile.TileContext(nc) as tc, nc.allow_low_precision(reason="bf16 matmul pipeline"):
        with (
            tc.tile_pool(name="persist", bufs=1) as persist,
            tc.tile_pool(name="consts", bufs=1) as consts,
        ):
            # Persistent SBUF tensors
            q_sb = persist.tile([128, FCH, S], BF16, tag="q_sb")
            k_sb = persist.tile([128, FCH, S], BF16, tag="k_sb")
            # V per head, with an appended ones column (65th) for denominators
            v_sb = [persist.tile([128, NKB, HD + 1], BF16, tag=f"v_sb{h}",
                                 name=f"v_sb{h}")
                    for h in range(HEADS_PER_CORE)]
            ot_sb = persist.tile([128, FCH, S], BF16, tag="ot_sb")

            cos_sb = consts.tile([128, S], BF16, tag="cos_sb")
            sin_sb = consts.tile([128, S], BF16, tag="sin_sb")
            mask_sb = consts.tile([128, 128], BF16, tag="mask_sb")
            bd1_sb = consts.tile([128, 2], BF16, tag="bd1_sb")
            qbd_sb = consts.tile([2, 128], BF16, tag="qbd_sb")
            kbd_sb = consts.tile([2, 128], BF16, tag="kbd_sb")
            rotm_sb = consts.tile([128, 128], BF16, tag="rotm_sb")
            sink_sb = consts.tile([1, HEADS_PER_CORE], F32, tag="sink_sb")
            ones_sb = consts.tile([1, HD], BF16, tag="ones_sb")
            ident_sb = consts.tile([128, 128], BF16, tag="ident_sb")
            eps_sb = consts.tile([128, 1], F32, tag="eps_sb")

            nc.gpsimd.dma_start(cos_sb[:], cosT)
            nc.gpsimd.dma_start(sin_sb[:], sinT)
            nc.gpsimd.dma_start(mask_sb[:], trimask)
            nc.gpsimd.dma_start(bd1_sb[:], bd1)
            nc.gpsimd.dma_start(qbd_sb[:], qbd)
            nc.gpsimd.dma_start(kbd_sb[:], kbd)
            nc.gpsimd.dma_start(rotm_sb[:], rotm)
            nc.gpsimd.dma_start(sink_sb[:], sinkexp)
            nc.vector.memset(ones_sb[:], 1.0)
            make_identity(nc, ident_sb[:])
            nc.vector.memset(eps_sb[:], EPS)
            for h in range(HEADS_PER_CORE):
                nc.vector.memset(v_sb[h][:, :, HD:HD + 1], 1.0)

            # PE warm-up: dense junk matmuls during the input DMA ramp so
            # the HAM clock gate reaches 2.4 GHz before real work arrives.
            with tc.tile_pool(name="warm", bufs=1) as warm, \
                 tc.tile_pool(name="warm_ps", bufs=1, space="PSUM") as warm_ps:
                wz = warm.tile([128, SQ], BF16, tag="wz")
                nc.vector.memset(wz[:], 0.0)
                wps = warm_ps.tile([128, SQ], F32, tag="wps")
                for i in range(48):
                    nc.tensor.matmul(wps[:], wz[:, 0:128], wz[:],
                                     start=True, stop=True)

            # ---------------- Phase 1: QKV projections -----------------
            with (
                tc.tile_pool(name="p1", bufs=2) as p1,
                tc.tile_pool(name="p1x", bufs=1) as p1x,
                tc.tile_pool(name="p1work", bufs=3) as p1w,
                tc.tile_pool(name="p1ps", bufs=4, space="PSUM") as p1ps,
                tc.tile_pool(name="p1ps_small", bufs=2, space="PSUM") as p1pss,
            ):
                xt_sb = p1x.tile([128, DCH, S], BF16, tag="xt_sb")
                wq_sb0 = p1.tile([128, DCH, FEATS], BF16, tag="w_sb", name="wq_sb")
                nc.sync.dma_start(wq_sb0[:], wqT3)
                for d in range(DCH):
                    nc.sync.dma_start(xt_sb[:, d, :], xT3[:, d, :])

                def qk_proj(wT3_ap, bd_w_sb, dst_sb, wtag):
                    if wT3_ap is None:
                        w_sb = wq_sb0
                    else:
                        w_sb = p1.tile([128, DCH, FEATS], BF16, tag="w_sb", name=wtag)
                        nc.sync.dma_start(w_sb[:], wT3_ap)
                    for f in range(FCH):
                        pss = []
                        for s in range(NSQ):
                            ps = p1ps.tile([128, SQ], F32, tag="qkv_ps")
                            pss.append(ps)
                        for d in range(DCH):
                            for s in range(NSQ):
                                nc.tensor.matmul(
                                    pss[s][:],
                                    w_sb[:, d, f * 128:(f + 1) * 128],
                                    xt_sb[:, d, s * SQ:(s + 1) * SQ],
                                    start=(d == 0),
                                    stop=(d == DCH - 1),
                                )
                        for s in range(NSQ):
                            ps = pss[s]
                            raw = p1w.tile([128, SQ], BF16, tag="raw")
                            sq = p1w.tile([128, SQ], BF16, tag="sq")
                            nc.scalar.copy(raw[:], ps[:])
                            nc.scalar.square(sq[:], ps[:])
                            ss_ps = p1pss.tile([128, SQ], F32, tag="sml_ps",
                                               name="ss_ps")[0:2]
                            nc.tensor.matmul(ss_ps[:], bd1_sb[:], sq[:],
                                             start=True, stop=True)
                            rms = p1w.tile([2, SQ], F32, tag="rms")
                            nc.scalar.activation(
                                rms[:], ss_ps[:], SQRT,
                                bias=eps_sb[0:2, :], scale=1.0 / HD,
                            )
                            rinv = p1w.tile([2, SQ], F32, tag="rinv")
                            nc.vector.reciprocal_approx_fast(rinv[:], rms[:])
                            rmsb = p1w.tile([2, SQ], BF16, tag="rmsb")
                            nc.vector.tensor_copy(rmsb[:], rinv[:])
                            bsc_ps = p1pss.tile([128, SQ], F32, tag="sml_ps",
                                                name="bsc_ps")
                            nc.tensor.matmul(bsc_ps[:], bd_w_sb[:], rmsb[:],
                                             start=True, stop=True)
                            qn = p1w.tile([128, SQ], BF16, tag="qn")
                            nc.vector.tensor_mul(qn[:], raw[:], bsc_ps[:])
                            # RoPE: rotate-half via 32-partition shifted copies
                            qs = p1w.tile([128, SQ], BF16, tag="qs")
                            nc.vector.tensor_mul(
                                qs[:], qn[:], sin_sb[:, s * SQ:(s + 1) * SQ])
                            rot = p1w.tile([128, SQ], BF16, tag="rot")
                            for base in (0, 64):
                                nc.vector.tensor_scalar_mul(
                                    rot[base:base + 32, :],
                                    qs[base + 32:base + 64, :], -1.0)
                                nc.vector.tensor_copy(
                                    rot[base + 32:base + 64, :],
                                    qs[base:base + 32, :])
                            qc = p1w.tile([128, SQ], BF16, tag="qc")
                            nc.vector.tensor_mul(
                                qc[:], qn[:], cos_sb[:, s * SQ:(s + 1) * SQ])
                            nc.vector.tensor_add(
                                dst_sb[:, f, s * SQ:(s + 1) * SQ], qc[:], rot[:])

                qk_proj(None, qbd_sb, q_sb, "wq_sb")
                qk_proj(wkT3, kbd_sb, k_sb, "wk_sb")

                # V projection: sequence-major, lhsT = xT tile (plain
                # matmuls only -- transpose-mode ops don't count as PE-busy
                # for the HAM clock gate and would cool the PE).
                wv_sb = p1.tile([128, DCH, FEATS], BF16, tag="w_sb", name="wv_sb")
                nc.sync.dma_start(wv_sb[:], wvT3)
                for g in range(NQT // 4):
                    pss = []
                    for qi in range(4):
                        ps = p1ps.tile([128, SQ], F32, tag="qkv_ps", name="v_ps")
                        pss.append(ps)
                    for d in range(DCH):
                        for qi in range(4):
                            qt = g * 4 + qi
                            nc.tensor.matmul(
                                pss[qi][:, :FEATS],
                                xt_sb[:, d, qt * 128:(qt + 1) * 128],
                                wv_sb[:, d, :],
                                start=(d == 0),
                                stop=(d == DCH - 1),
                            )
                    for qi in range(4):
                        qt = g * 4 + qi
                        for h in range(HEADS_PER_CORE):
                            nc.scalar.copy(
                                v_sb[h][:, qt, 0:HD],
                                pss[qi][:, h * HD:(h + 1) * HD])

            # ------------- Phase 2+3: attention + output projection ------
            # t (512-wide q-range) is the outer loop; once both head-pair
            # chunks finish a q-range, its slice of the output projection is
            # issued, overlapping the next tile's attention and keeping the
            # PE fed with full-array matmuls.
            with (
                tc.tile_pool(name="p2p", bufs=4) as p2p,
                tc.tile_pool(name="p2w", bufs=2) as p2w,
                tc.tile_pool(name="p3", bufs=1) as p3,
                tc.tile_pool(name="p3w", bufs=3) as p3w,
                tc.tile_pool(name="p2s_ps", bufs=2, space="PSUM") as p2sps,
                tc.tile_pool(name="p2o_ps", bufs=3, space="PSUM") as p2ops,
                tc.tile_pool(name="p3ps", bufs=1, space="PSUM") as p3ps,
            ):
                wo_sb = p3.tile([128, FCH, D], BF16, tag="wo_sb")
                nc.gpsimd.dma_start(wo_sb[:], woT3)
                for t in range(NSQ):
                    nkb = (t + 1) * (SQ // 128)  # causal: key blocks 0..nkb-1
                    for f in range(FCH):
                        ot_ps = [
                            p2ops.tile([HD + 1, SQ], F32, tag="ot_ps",
                                       name=f"ot_ps{hh}")
                            for hh in range(2)
                        ]
                        for kb in range(nkb):
                            j = kb - (t * (SQ // 128))
                            qlo = max(j, 0) * 128
                            sp = p2sps.tile([128, 2 * SQ], F32, tag="s_ps")
                            for hh in range(2):
                                plo = hh * HD
                                nc.tensor.matmul(
                                    sp[:, hh * SQ + qlo:(hh + 1) * SQ],
                                    k_sb[plo:plo + HD, f, kb * 128:(kb + 1) * 128],
                                    q_sb[plo:plo + HD, f, t * SQ + qlo:(t + 1) * SQ],
                                    start=True, stop=True,
                                )
                            p_sb = p2p.tile([128, 2 * SQ], BF16, tag="p_sb")
                            if qlo == 0:
                                nc.scalar.activation(p_sb[:], sp[:], EXP)
                            else:
                                for hh in range(2):
                                    nc.scalar.activation(
                                        p_sb[:, hh * SQ + qlo:(hh + 1) * SQ],
                                        sp[:, hh * SQ + qlo:(hh + 1) * SQ], EXP)
                            if j >= 0:
                                for hh in range(2):
                                    nc.vector.tensor_mul(
                                        p_sb[:, hh * SQ + qlo:hh * SQ + qlo + 128],
                                        p_sb[:, hh * SQ + qlo:hh * SQ + qlo + 128],
                                        mask_sb[:])
                            for hh in range(2):
                                h = 2 * f + hh
                                nc.tensor.matmul(
                                    ot_ps[hh][:, qlo:],
                                    v_sb[h][:, kb, :],
                                    p_sb[:, hh * SQ + qlo:(hh + 1) * SQ],
                                    start=(kb == 0),
                                    stop=(kb == nkb - 1),
                                )
                        # denominators: row HD of ot_ps (+ exp(sink)).  The two
                        # copies run first so ot_ps frees for the next tile.
                        dens, ot_us = [], []
                        for hh in range(2):
                            den = p2w.tile([1, SQ], F32, tag="den",
                                           name=f"den{hh}")
                            nc.vector.tensor_copy(den[:], ot_ps[hh][HD:HD + 1, :])
                            ot_u = p2w.tile([HD, SQ], BF16, tag="ot_u",
                                            name=f"ot_u{hh}")
                            nc.vector.tensor_copy(ot_u[:], ot_ps[hh][0:HD, :])
                            dens.append(den)
                            ot_us.append(ot_u)
                        for hh in range(2):
                            h = 2 * f + hh
                            plo = hh * HD
                            den, ot_u = dens[hh], ot_us[hh]
                            nc.vector.tensor_scalar_add(
                                den[:], den[:], sink_sb[0:1, h:h + 1])
                            nc.vector.reciprocal_approx_fast(den[:], den[:])
                            bc_sb = p2w.tile([HD, SQ], F32, tag="bc_sb",
                                             name=f"bc_sb{hh}")
                            nc.gpsimd.partition_broadcast(bc_sb[:], den[:])
                            nc.vector.tensor_mul(
                                ot_sb[plo:plo + HD, f, t * SQ:(t + 1) * SQ],
                                ot_u[:], bc_sb[:])
                    # output projection for this q-range (4 x 128 rows)
                    for qi in range(SQ // 128):
                        qt = t * (SQ // 128) + qi
                        y_sb = p3w.tile([128, D], F32, tag="y_sb")
                        for n in range(D // SQ):
                            y_ps = p3ps.tile([128, SQ], F32, tag="y_ps")
                            for c in range(FCH):
                                nc.tensor.matmul(
                                    y_ps[:],
                                    ot_sb[:, c, qt * 128:(qt + 1) * 128],
                                    wo_sb[:, c, n * SQ:(n + 1) * SQ],
                                    start=(c == 0),
                                    stop=(c == FCH - 1),
                                )
                            nc.scalar.copy(y_sb[:, n * SQ:(n + 1) * SQ], y_ps[:])
                        nc.sync.dma_start(y[qt * 128:(qt + 1) * 128, :], y_sb[:])

    nc.compile()
    return nc


_NC_CACHE = None


def _get_program():
    global _NC_CACHE
    if _NC_CACHE is None:
        _NC_CACHE = build_program()
    return _NC_CACHE


def _b(x):
    return np.ascontiguousarray(np.asarray(x, dtype=np.float32)).astype(BF16_NP)


def _host_inputs(x, wq, wk, wv, wo, q_norm_w, k_norm_w, sink_logit):
    """Build the 8 per-core input maps."""
    x = np.asarray(x, dtype=np.float32)
    wq = np.asarray(wq, dtype=np.float32)
    wk = np.asarray(wk, dtype=np.float32)
    wv = np.asarray(wv, dtype=np.float32)
    wo = np.asarray(wo, dtype=np.float32)
    q_norm_w = np.asarray(q_norm_w, dtype=np.float32)
    k_norm_w = np.asarray(k_norm_w, dtype=np.float32)
    sink_logit = np.asarray(sink_logit, dtype=np.float32)

    # rope tables, feature-major, duplicated across the two heads per chunk
    inv_freq = 1.0 / (ROPE_BASE ** (np.arange(0, HD, 2, dtype=np.float32) / HD))
    tpos = np.arange(S, dtype=np.float32)
    freqs = tpos[:, None] * inv_freq[None, :]           # [S, 32]
    emb = np.concatenate([freqs, freqs], axis=-1)       # [S, 64]
    cosT = _b(np.tile(np.cos(emb).T, (2, 1)))           # [128, S]
    sinT = _b(np.tile(np.sin(emb).T, (2, 1)))

    # triangular causal mask for the single diagonal 128-wide band
    kk = np.arange(128)[:, None]
    qq = np.arange(128)[None, :]
    trimask = _b((kk <= qq).astype(np.float32))          # [128, 128]

    bd1 = np.zeros((128, 2), dtype=np.float32)
    bd1[0:64, 0] = 1.0
    bd1[64:128, 1] = 1.0
    bd1 = _b(bd1)

    scale_half = float(HD) ** -0.25  # sqrt of softmax scale, folded into q & k
    qbd = np.zeros((2, 128), dtype=np.float32)
    kbd = np.zeros((2, 128), dtype=np.float32)
    for m in range(2):
        qbd[m, m * 64:(m + 1) * 64] = q_norm_w * scale_half
        kbd[m, m * 64:(m + 1) * 64] = k_norm_w * scale_half
    qbd = _b(qbd)
    kbd = _b(kbd)

    rotm = np.zeros((128, 128), dtype=np.float32)
    for base in (0, 64):
        for m in range(32):
            rotm[base + m + 32, base + m] = -1.0
            rotm[base + m, base + m + 32] = 1.0
    rotm = _b(rotm)

    in_maps = []
    xT_b = [_b(x[b].T) for b in range(B)]
    for core in range(N_CORES):
        b = core // 4
        hg = core % 4
        rows = slice(hg * FEATS, (hg + 1) * FEATS)
        heads = slice(hg * HEADS_PER_CORE, (hg + 1) * HEADS_PER_CORE)
        in_maps.append({
            "xT": xT_b[b],
            "wqT": _b(wq[rows, :].T),
            "wkT": _b(wk[rows, :].T),
            "wvT": _b(wv[rows, :].T),
            "woT": _b(wo[:, rows].T),
            "cosT": cosT,
            "sinT": sinT,
            "trimask": trimask,
            "bd1": bd1,
            "qbd": qbd,
            "kbd": kbd,
            "rotm": rotm,
            "sinkexp": np.exp(sink_logit[heads]).astype(np.float32).reshape(
                1, HEADS_PER_CORE),
        })
    return in_maps


def kernel(x, wq, wk, wv, wo, q_norm_w, k_norm_w, sink_logit, _run_kwargs=None):
    nc = _get_program()
    in_maps = _host_inputs(x, wq, wk, wv, wo, q_norm_w, k_norm_w, sink_logit)
    res = run_bass_kernel_spmd(nc, in_maps, core_ids=list(range(N_CORES)),
                               **(_run_kwargs or {}))
    out = np.zeros((B, S, D), dtype=np.float32)
    for core in range(N_CORES):
        out[core // 4] += res.results[core]["y"]
    if _run_kwargs:
        kernel.last_result = res
    return out
